# revision 1
# baseline (speedup 1.0000x reference)
"""Trainium2 Bass kernel for nn_CollisonToJointLoss.

Math restructure (avoids the 3K*J*J elementwise blowup):
  joint_regressor >= 0, so where both gathered scores are nonzero,
  |intr_s + recv_s| = intr_s + recv_s.  For one collision-vertex pair
  (u = intruder vertex, w = receiver vertex, batch b):

    sum_{i,j} mask*d*(s_u[i]+s_w[j]) = s_u^T D_b m_w + m_u^T D_b s_w
    sum_{i,j} mask                   = m_u^T (11^T) m_w

  with s_v = jr[v], m_v = (s_v > 0), D_b = pairwise joint distances.
  Summing over pairs becomes one accumulated PE matmul per batch with
  rows [S|M]: ACC = [S|M]_intr^T @ [S|M]_recv, where
    TR = S_i^T M_r, BL = M_i^T S_r (the two num terms), BR = M_i^T M_r (den).
  num_b = <D_b, TR + BL>, den_b = sum(BR).

Gathers run via gpsimd.dma_gather (256B rows, int16 wrapped indices).
The wrapped index layout (idx r at partition r%16 replicated across
16-partition groups, slot r//16) is built with one contiguous
SBUF->DRAM write plus 8 per-group loads; slot assignments are chosen
so each collision's intruder/receiver rows land partition-aligned for
the matmul (side distance in the slot dimension divisible by 8).

Invalid collisions (cf[:,0] < 0) redirect both face ids to a padded
zero-face whose vertices point at an all-zero jr row => S = M = 0.

Sharding: data-parallel over batch B: 8 cores x 2 batches.  Each core
returns partial (num, den); host sums and finishes the mean.
"""

import numpy as np

B, C, N, F, J = 16, 2048, 6890, 13776, 24
NCORES = 8
BPC = B // NCORES          # batches per core
NPAD = 6912                # 128 * 54  (jr/verts padded with zero rows)
KCH = NPAD // 128          # 54 chunks for the joints matmul
CPP = C // 128             # 16 collisions per partition per batch
FPAD = F + 1               # faces rows per batch incl. zero-face row
J2 = 2 * J                 # 48
E = 64                     # gather row width (256B granularity)

_CACHE = {}


def _build_program():
    import concourse.bass as bass
    import concourse.tile as tile
    from concourse import bacc, mybir
    from concourse.masks import make_identity

    f32 = mybir.dt.float32
    i32 = mybir.dt.int32
    i16 = mybir.dt.int16
    Alu = mybir.AluOpType

    nc = bacc.Bacc("TRN2", target_bir_lowering=False, debug=False,
                   num_swdge_queues=2)

    jrt = nc.dram_tensor("jrt", [NPAD, J], f32, kind="ExternalInput").ap()
    verts = nc.dram_tensor("verts", [BPC, NPAD, 3], f32, kind="ExternalInput").ap()
    cf = nc.dram_tensor("cf", [BPC, C, 2], i32, kind="ExternalInput").ap()
    faces64 = nc.dram_tensor("faces64", [BPC * FPAD, E], i32,
                             kind="ExternalInput").ap()
    out_d = nc.dram_tensor("out", [2, 1], f32, kind="ExternalOutput").ap()

    with tile.TileContext(nc) as tc:
        with tc.tile_pool(name="sb", bufs=1) as sb, \
             tc.tile_pool(name="pp", bufs=1, space="PSUM") as pp, \
             tc.tile_pool(name="dp", bufs=1, space="DRAM") as dp:

            # ---- bulk loads -------------------------------------------------
            JT = sb.tile([128, KCH, J], f32)        # jr rows 54p+k at (p, k)
            nc.sync.dma_start(out=JT[:].rearrange("p k j -> p (k j)"),
                              in_=jrt.rearrange("(p k) j -> p (k j)", p=128))

            VT0 = sb.tile([128, KCH, 3], f32)
            VT1 = sb.tile([128, KCH, 3], f32)
            nc.sync.dma_start(out=VT0[:].rearrange("p k d -> p (k d)"),
                              in_=verts[0].rearrange("(p k) d -> p (k d)", p=128))
            nc.sync.dma_start(out=VT1[:].rearrange("p k d -> p (k d)"),
                              in_=verts[1].rearrange("(p k) d -> p (k d)", p=128))
            VC = sb.tile([128, KCH, 6], f32)
            nc.vector.tensor_copy(out=VC[:, :, 0:3], in_=VT0[:])
            nc.vector.tensor_copy(out=VC[:, :, 3:6], in_=VT1[:])

            CFt = sb.tile([128, BPC * CPP * 2], i32)
            for b in range(BPC):
                nc.sync.dma_start(
                    out=CFt[:, b * CPP * 2:(b + 1) * CPP * 2],
                    in_=cf[b].rearrange("(p n) s -> p (n s)", p=128))

            ident = sb.tile([128, 128], f32)
            make_identity(nc, ident[:])

            # ---- [S | M | pad] gather table in DRAM -------------------------
            TSM = sb.tile([128, KCH, E], f32)
            nc.vector.tensor_copy(out=TSM[:, :, 0:J], in_=JT[:])
            nc.vector.tensor_scalar(out=TSM[:, :, J:J2], in0=JT[:],
                                    scalar1=0.0, scalar2=None, op0=Alu.is_gt)
            nc.vector.memset(TSM[:, :, J2:E], 0.0)
            tsm = dp.tile([NPAD, E], f32)
            nc.sync.dma_start(out=tsm[:].rearrange("(p k) j -> p (k j)", p=128),
                              in_=TSM[:].rearrange("p k j -> p (k j)"))

            # ---- joints = jr^T-chunks contracted with verts ------------------
            J6p = pp.tile([J, 6], f32)
            for k in range(KCH):
                nc.tensor.matmul(out=J6p[:], lhsT=JT[:, k, :], rhs=VC[:, k, :],
                                 start=(k == 0), stop=(k == KCH - 1))
            j6 = sb.tile([J, 6], f32)
            nc.vector.tensor_copy(out=j6[:], in_=J6p[:])
            j6sq = sb.tile([J, 6], f32)
            nc.vector.tensor_mul(out=j6sq[:], in0=j6[:], in1=j6[:])

            ones3_24 = sb.tile([3, J], f32)
            nc.vector.memset(ones3_24[:], 1.0)
            ones24 = sb.tile([J, 1], f32)
            nc.vector.memset(ones24[:], 1.0)

            # jt_b = joints_b^T [3, 24] at partition base 0
            jtp = pp.tile([3, J2], f32)
            for b in range(BPC):
                nc.tensor.transpose(out=jtp[:, J * b:J * b + J],
                                    in_=j6[:, 3 * b:3 * b + 3],
                                    identity=ident[:J, :J])
            jt = sb.tile([3, J2], f32)
            nc.vector.tensor_copy(out=jt[:], in_=jtp[:])
            sq = sb.tile([3, J2], f32)
            nc.vector.tensor_mul(out=sq[:], in0=jt[:], in1=jt[:])
            jtm2 = sb.tile([3, J2], f32)
            nc.vector.tensor_scalar_mul(out=jtm2[:], in0=jt[:], scalar1=-2.0)

            # ---- D_b = pairwise joint distances (partitions 0-23) -----------
            G24 = pp.tile([J, J], f32)
            D_b = [sb.tile([J, J], f32, name=f"D{b}") for b in range(BPC)]
            rc = sb.tile([J, BPC], f32)
            for b in range(BPC):
                nc.vector.reduce_sum(out=rc[:, b:b + 1], in_=j6sq[:, 3 * b:3 * b + 3],
                                     axis=mybir.AxisListType.X)
            for b in range(BPC):
                nc.tensor.matmul(out=G24[:], lhsT=jtm2[:, J * b:J * b + J],
                                 rhs=jt[:, J * b:J * b + J], start=True, stop=False)
                nc.tensor.matmul(out=G24[:], lhsT=ones3_24[:],
                                 rhs=sq[:, J * b:J * b + J], start=False, stop=True)
                nc.vector.tensor_scalar(out=D_b[b][:], in0=G24[:],
                                        scalar1=rc[:, b:b + 1], scalar2=0.0,
                                        op0=Alu.add, op1=Alu.max)
                nc.scalar.activation(out=D_b[b][:], in_=D_b[b][:],
                                     func=mybir.ActivationFunctionType.Sqrt)

            # ---- collision index processing (both batches) ------------------
            CFv = CFt[:].rearrange("p (b n s) -> p b n s", b=BPC, s=2)
            CFf = sb.tile([128, BPC, CPP, 2], f32)
            nc.vector.tensor_copy(out=CFf[:], in_=CFv)
            V0 = sb.tile([128, BPC, CPP, 1], f32)
            nc.vector.tensor_scalar(out=V0[:], in0=CFf[:, :, :, 0:1],
                                    scalar1=0.0, scalar2=None, op0=Alu.is_ge)
            nc.vector.tensor_scalar_max(out=CFf[:], in0=CFf[:], scalar1=0.0)
            # sel = valid ? cf : F  (zero-face row in faces64_b)
            T1 = sb.tile([128, BPC, CPP, 2], f32)
            nc.vector.tensor_tensor(out=T1[:], in0=CFf[:],
                                    in1=V0[:].to_broadcast([128, BPC, CPP, 2]),
                                    op=Alu.mult)
            T2 = sb.tile([128, BPC, CPP, 1], f32)
            nc.vector.tensor_scalar(out=T2[:], in0=V0[:],
                                    scalar1=-float(F), scalar2=float(F),
                                    op0=Alu.mult, op1=Alu.add)
            nc.vector.tensor_tensor(out=T1[:], in0=T1[:],
                                    in1=T2[:].to_broadcast([128, BPC, CPP, 2]),
                                    op=Alu.add)
            for b in range(1, BPC):
                nc.vector.tensor_scalar_add(out=T1[:, b], in0=T1[:, b],
                                            scalar1=float(b * FPAD))

            # ---- merged gather pipeline (both batches in one shot) ----------
            NFI = BPC * CPP * 2 * 128      # 8192 face indices
            NJI = BPC * CPP * 2 * 3 * 128  # 24576 jr indices
            ACC = [pp.tile([J2, J2], f32, name=f"ACC{b}") for b in range(BPC)]
            # face ids int16, slot cF = 32b + 16s + n
            CFb16 = sb.tile([128, NFI // 128], i16)
            nc.vector.tensor_copy(
                out=CFb16[:].rearrange("p (b s n) -> p b s n", b=BPC, s=2),
                in_=T1[:].rearrange("p b n s -> p b s n"))
            # wrap-write: dramF[q16, phi, cF] = CFb16[16phi+q16, cF]
            dramF = dp.tile([16, 8, NFI // 128], i16)
            nc.sync.dma_start(out=dramF[:].rearrange("q phi c -> phi q c"),
                              in_=CFb16[:])
            WF = sb.tile([128, NFI // 16], i16)
            for g in range(8):
                nc.sync.dma_start(out=WF[16 * g:16 * (g + 1), :],
                                  in_=dramF[:].rearrange("q phi c -> q (phi c)"))
            # face gather: r=(64phi+cF)*16+q16 -> VB[16(cF%8)+q16, 8phi+cF//8]
            VB = sb.tile([128, NFI // 128, E], i32)
            nc.gpsimd.dma_gather(out_ap=VB[:], in_ap=faces64,
                                 idxs_ap=WF[:], num_idxs=NFI,
                                 num_idxs_reg=NFI, elem_size=E,
                                 single_packet=False)
            # vertex ids -> int16, mm = 96b + 48s + 6phi + 3kap + t
            # (c_vb = 8phi + 4b + 2s + kap; split by s to stay <=4 AP dims)
            VB16 = sb.tile([128, NJI // 128], i16)
            for s in range(2):
                vbv = VB[:, :, 0:3].rearrange("p (phi b s k) t -> p s b phi k t",
                                              phi=8, b=BPC, s=2)
                nc.vector.tensor_copy(
                    out=VB16[:].rearrange("p (b s phi k t) -> p s b phi k t",
                                          b=BPC, s=2, phi=8, k=2)[:, s],
                    in_=vbv[:, s])
            # wrap-write per batch: dramJ[b, q16, phi2, mmb]
            #   = VB16[16phi2+q16, 96b+mmb]
            NJB = NJI // BPC               # 12288 jr indices per batch
            dramJ = dp.tile([BPC, 16, 8, 96], i16)
            for b in range(BPC):
                nc.sync.dma_start(
                    out=dramJ[b].rearrange("q phi m -> phi q m"),
                    in_=VB16[:, b * 96:(b + 1) * 96])
            WJ = sb.tile([128, NJI // 16], i16)
            for g in range(8):
                nc.sync.dma_start(
                    out=WJ[16 * g:16 * (g + 1), :],
                    in_=dramJ[:].rearrange("b q phi m -> q b (phi m)"))
            # jr gather per batch (SWDGE ring caps ~1024 descriptors):
            # local r2=(96phi2+mmb)*16+q16 -> U[16(mmb%8)+q16, 96b+12phi2+mmb//8]
            U = sb.tile([128, NJI // 128, E], f32)
            for b in range(BPC):
                nc.gpsimd.dma_gather(
                    out_ap=U[:, b * 96:(b + 1) * 96, :], in_ap=tsm[:],
                    idxs_ap=WJ[:, b * (NJB // 16):(b + 1) * (NJB // 16)],
                    num_idxs=NJB, num_idxs_reg=NJB, elem_size=E,
                    single_packet=False, queue_num=b)

            # ---- main accumulation ------------------------------------------
            for b in range(BPC):
                first = True
                for nu in range(8):
                    for mu in range(6):
                        c = 96 * b + 12 * nu + mu
                        nc.tensor.matmul(out=ACC[b][:],
                                         lhsT=U[:, c + 6, 0:J2],
                                         rhs=U[:, c, 0:J2],
                                         start=first,
                                         stop=(nu == 7 and mu == 5))
                        first = False

            # ---- final reductions -------------------------------------------
            # BRBL = ACC[24:48, :] moved to partitions 0-23 via selector matmul
            OUT2 = pp.tile([2, 1], f32)
            BRBL = pp.tile([J, J2], f32)
            Vc = [sb.tile([J, 2], f32, name=f"Vc{b}") for b in range(BPC)]
            for b in range(BPC):
                ACCs = sb.tile([J2, J2], f32, name=f"ACCs{b}")
                nc.vector.tensor_copy(out=ACCs[:], in_=ACC[b][:])
                nc.tensor.matmul(out=BRBL[:], lhsT=ident[0:J2, J:J2],
                                 rhs=ACCs[:], start=True, stop=True)
                # num terms: TR = ACCs[0:24, 24:48], BL = BRBL[:, 0:24]
                NU = sb.tile([J, J2], f32, name=f"NU{b}")
                nc.vector.tensor_mul(out=NU[:, 0:J], in0=ACCs[0:J, J:J2],
                                     in1=D_b[b][:])
                nc.vector.tensor_mul(out=NU[:, J:J2], in0=BRBL[:, 0:J],
                                     in1=D_b[b][:])
                nc.vector.reduce_sum(out=Vc[b][:, 0:1], in_=NU[:],
                                     axis=mybir.AxisListType.X)
                nc.vector.reduce_sum(out=Vc[b][:, 1:2], in_=BRBL[:, J:J2],
                                     axis=mybir.AxisListType.X)
            for b in range(BPC):
                nc.tensor.matmul(out=OUT2[:], lhsT=Vc[b][:], rhs=ones24[:],
                                 start=(b == 0), stop=(b == BPC - 1))

            outs = sb.tile([2, 1], f32)
            nc.vector.tensor_copy(out=outs[:], in_=OUT2[:])
            nc.sync.dma_start(out=out_d, in_=outs[:])

    nc.compile()
    return nc


def get_program():
    if "nc" not in _CACHE:
        _CACHE["nc"] = _build_program()
    return _CACHE["nc"]


def make_in_maps(collision_idxs, vertices, faces, joint_regressor):
    """Host-side shard/layout prep. Returns list of per-core input dicts."""
    collision_idxs = np.asarray(collision_idxs)
    vertices = np.asarray(vertices)
    faces = np.asarray(faces)
    joint_regressor = np.asarray(joint_regressor)
    jrt = np.zeros((NPAD, J), dtype=np.float32)
    jrt[:N, :] = np.ascontiguousarray(joint_regressor.T.astype(np.float32))

    vpad = np.zeros((B, NPAD, 3), dtype=np.float32)
    vpad[:, :N, :] = vertices.astype(np.float32)

    cfi = collision_idxs.astype(np.int32)
    # faces rows embedded in 256B rows; final row = zero-face -> zero jr row
    f64 = np.zeros((B, FPAD, E), dtype=np.int32)
    f64[:, :F, 0:3] = faces.astype(np.int32)
    f64[:, F, 0:3] = N

    in_maps = []
    for c in range(NCORES):
        bs = slice(c * BPC, (c + 1) * BPC)
        m = {
            "jrt": jrt,
            "verts": np.ascontiguousarray(vpad[bs]),
            "cf": np.ascontiguousarray(cfi[bs]),
        }
        m["faces64"] = np.ascontiguousarray(
            f64[bs].reshape(BPC * FPAD, E))
        in_maps.append(m)
    return in_maps


def kernel(collision_idxs, vertices, faces, joint_regressor):
    from concourse.bass_utils import run_bass_kernel_spmd

    nc = get_program()
    in_maps = make_in_maps(collision_idxs, vertices, faces, joint_regressor)
    res = run_bass_kernel_spmd(nc, in_maps, core_ids=list(range(NCORES)))
    num = 0.0
    den = 0.0
    for r in res.results:
        o = np.asarray(r["out"], dtype=np.float64).reshape(-1)
        num += o[0]
        den += o[1]
    if den > 0:
        val = num / max(den, 1.0)
    else:
        val = 0.0
    return np.float32(val)



# revision 2
# speedup vs baseline: 4.4630x; 4.4630x over previous
"""Trainium2 Bass kernel for nn_CollisonToJointLoss.

Math restructure (avoids the 3K*J*J elementwise blowup):
  joint_regressor >= 0, so where both gathered scores are nonzero,
  |intr_s + recv_s| = intr_s + recv_s.  For one collision-vertex pair
  (u = intruder vertex, w = receiver vertex, batch b):

    sum_{i,j} mask*d*(s_u[i]+s_w[j]) = s_u^T D_b m_w + m_u^T D_b s_w
    sum_{i,j} mask                   = m_u^T (11^T) m_w
  with s_v = jr[v], m_v = (s_v > 0), D_b = pairwise joint distances.
  Summing over pairs becomes accumulated PE matmuls with rows [S|M]:
  ACC = [S|M]_intr^T @ [S|M]_recv; TR = S_i^T M_r, BL = M_i^T S_r,
  BR = M_i^T M_r.  num_b = <D_b, TR + BL>, den_b = sum(BR).

Single-stage gather: the host joins faces with the jr score table into a
per-(batch,face) row TF[b*FPAD+f] = [T(v0)|T(v1)|T(v2)|pad] (bf16,
512B rows, T(v) = [S(v)(24)|M(v)(24)]), with an all-zero row at f=F for
invalid collisions.  One descriptor per collision side (2*C per batch =
8192 per core) fetches matmul-ready rows, vs 4 face + 24 jr descriptors
per collision in a two-stage design.  Gather indices are pure index
arithmetic on collision_idxs, so the host computes them in the wrapped
int16 layout dma_gather wants; the kernel just loads them.

The gather is split into 4 calls (desc-gen pipelines with DMA); each
call delivers complete intr/recv pairs for 1/4 of both batches'
collisions, so ACC matmuls stream behind the gather DMA.

Sharding: data-parallel over batch B: 8 cores x 2 batches.  Each core
returns partial (num, den); host sums and finishes the mean.
"""

import numpy as np

B, C, N, F, J = 16, 2048, 6890, 13776, 24
NCORES = 8
BPC = B // NCORES          # batches per core
NPAD = 6912                # 128 * 54  (jr/verts padded with zero rows)
KCH = NPAD // 128          # 54 chunks for the joints matmul
FPAD = F + 1               # TF rows per batch incl. zero row
J2 = 2 * J                 # 48
EROW = 256                 # TF row width in bf16 elems (512B)
NQ = 4                     # gather split
NIDX = BPC * C * 2         # 8192 gather descriptors per core
NPQ = NIDX // NQ           # 2048 idxs per gather call
CHQ = C // 128 // NQ       # collision chunks per call per (batch,side) = 4

_CACHE = {}


def _build_program():
    import concourse.bass as bass
    import concourse.tile as tile
    from concourse import bacc, mybir
    from concourse.masks import make_identity

    f32 = mybir.dt.float32
    bf16 = mybir.dt.bfloat16
    i16 = mybir.dt.int16
    Alu = mybir.AluOpType

    nc = bacc.Bacc("TRN2", target_bir_lowering=False, debug=False,
                   num_swdge_queues=2)

    tf = nc.dram_tensor("tf", [BPC * FPAD, EROW], bf16,
                        kind="ExternalInput").ap()
    wf = nc.dram_tensor("wf", [128, NIDX // 16], i16,
                        kind="ExternalInput").ap()
    jr16 = nc.dram_tensor("jr16", [NPAD, J], bf16, kind="ExternalInput").ap()
    vp6 = nc.dram_tensor("vp6", [NPAD, 6], bf16, kind="ExternalInput").ap()
    out_d = nc.dram_tensor("out", [4, 1], f32, kind="ExternalOutput").ap()

    with tile.TileContext(nc) as tc:
        with tc.tile_pool(name="sb", bufs=1) as sb, \
             tc.tile_pool(name="pp", bufs=1, space="PSUM") as pp:

            # ---- loads (WF first: it gates the gather chain) --------------
            WF = sb.tile([128, NIDX // 16], i16)
            nc.sync.dma_start(out=WF[:], in_=wf)

            JT = sb.tile([128, KCH, J], bf16)        # jr row 54p+k at (p,k)
            nc.sync.dma_start(out=JT[:].rearrange("p k j -> p (k j)"),
                              in_=jr16.rearrange("(p k) j -> p (k j)", p=128))
            VP = sb.tile([128, KCH, 6], bf16)        # verts b0|b1 interleaved
            nc.sync.dma_start(out=VP[:].rearrange("p k d -> p (k d)"),
                              in_=vp6.rearrange("(p k) d -> p (k d)", p=128))

            ident = sb.tile([128, 128], f32)
            make_identity(nc, ident[:])

            # ---- gather: 4 pipelined calls, 2048 descriptors each ---------
            U = sb.tile([128, NQ * 16, EROW], bf16)
            for q in range(NQ):
                nc.gpsimd.dma_gather(
                    out_ap=U[:, q * 16:(q + 1) * 16, :], in_ap=tf,
                    idxs_ap=WF[:, q * (NPQ // 16):(q + 1) * (NPQ // 16)],
                    num_idxs=NPQ, num_idxs_reg=NPQ, elem_size=EROW,
                    single_packet=False, queue_num=q % 2)

            # ---- ACC matmuls stream behind each gather call ---------------
            # U c-slot = q*16 + (b*2+s)*4 + nn; collision chunk n = 4q+nn
            ACC = [pp.tile([J2, J2], f32, name=f"ACC{b}") for b in range(BPC)]
            for q in range(NQ):
                for b in range(BPC):
                    for nn in range(CHQ):
                        ci = q * 16 + (b * 2 + 1) * CHQ + nn
                        cr = q * 16 + (b * 2 + 0) * CHQ + nn
                        for t in range(3):
                            nc.tensor.matmul(
                                out=ACC[b][:],
                                lhsT=U[:, ci, t * J2:(t + 1) * J2],
                                rhs=U[:, cr, t * J2:(t + 1) * J2],
                                start=(q == 0 and nn == 0 and t == 0),
                                stop=(q == NQ - 1 and nn == CHQ - 1 and t == 2))

            # ---- joints = jr^T-chunks contracted with verts ----------------
            J6p = pp.tile([J, 6], f32)
            for k in range(KCH):
                nc.tensor.matmul(out=J6p[:], lhsT=JT[:, k, :], rhs=VP[:, k, :],
                                 start=(k == 0), stop=(k == KCH - 1))
            j6 = sb.tile([J, 6], f32)
            nc.vector.tensor_copy(out=j6[:], in_=J6p[:])
            j6sq = sb.tile([J, 6], f32)
            nc.vector.tensor_mul(out=j6sq[:], in0=j6[:], in1=j6[:])

            ones3_24 = sb.tile([3, J], f32)
            nc.vector.memset(ones3_24[:], 1.0)
            ones24 = sb.tile([J, 1], f32)
            nc.vector.memset(ones24[:], 1.0)

            # jt_b = joints_b^T [3, 24] at partition base 0
            jtp = pp.tile([3, J2], f32)
            for b in range(BPC):
                nc.tensor.transpose(out=jtp[:, J * b:J * b + J],
                                    in_=j6[:, 3 * b:3 * b + 3],
                                    identity=ident[:J, :J])
            jt = sb.tile([3, J2], f32)
            nc.vector.tensor_copy(out=jt[:], in_=jtp[:])
            sq = sb.tile([3, J2], f32)
            nc.vector.tensor_mul(out=sq[:], in0=jt[:], in1=jt[:])
            jtm2 = sb.tile([3, J2], f32)
            nc.vector.tensor_scalar_mul(out=jtm2[:], in0=jt[:], scalar1=-2.0)

            # ---- D_b = pairwise joint distances (partitions 0-23) ---------
            G24 = pp.tile([J, J], f32)
            D_b = [sb.tile([J, J], f32, name=f"D{b}") for b in range(BPC)]
            rc = sb.tile([J, BPC], f32)
            for b in range(BPC):
                nc.vector.reduce_sum(out=rc[:, b:b + 1],
                                     in_=j6sq[:, 3 * b:3 * b + 3],
                                     axis=mybir.AxisListType.X)
            for b in range(BPC):
                nc.tensor.matmul(out=G24[:], lhsT=jtm2[:, J * b:J * b + J],
                                 rhs=jt[:, J * b:J * b + J],
                                 start=True, stop=False)
                nc.tensor.matmul(out=G24[:], lhsT=ones3_24[:],
                                 rhs=sq[:, J * b:J * b + J],
                                 start=False, stop=True)
                nc.vector.tensor_scalar(out=D_b[b][:], in0=G24[:],
                                        scalar1=rc[:, b:b + 1], scalar2=0.0,
                                        op0=Alu.add, op1=Alu.max)
                nc.scalar.activation(out=D_b[b][:], in_=D_b[b][:],
                                     func=mybir.ActivationFunctionType.Sqrt)

            # ---- final reductions ----------------------------------------
            # ACCs = [ACC0 | ACC1] [48, 96]; BRBL = rows 24:48 -> parts 0:23
            ACCs = sb.tile([J2, BPC * J2], f32)
            for b in range(BPC):
                nc.vector.tensor_copy(out=ACCs[:, b * J2:(b + 1) * J2],
                                      in_=ACC[b][:])
            BRBLp = pp.tile([J, BPC * J2], f32)
            nc.tensor.matmul(out=BRBLp[:], lhsT=ident[0:J2, J:J2],
                             rhs=ACCs[:], start=True, stop=True)
            BRBL = sb.tile([J, BPC * J2], f32)
            nc.vector.tensor_copy(out=BRBL[:], in_=BRBLp[:])

            # Vc cols: (num_b contributions, den_b) per batch
            NU = sb.tile([J, BPC * J2], f32)
            Vc = sb.tile([J, 2 * BPC], f32)
            for b in range(BPC):
                # TR = ACCs[0:24, b*48+24 : b*48+48], BL = BRBL[:, b*48 : b*48+24]
                nc.vector.tensor_mul(out=NU[:, b * J2:b * J2 + J],
                                     in0=ACCs[0:J, b * J2 + J:(b + 1) * J2],
                                     in1=D_b[b][:])
                nc.vector.tensor_mul(out=NU[:, b * J2 + J:(b + 1) * J2],
                                     in0=BRBL[:, b * J2:b * J2 + J],
                                     in1=D_b[b][:])
                nc.vector.reduce_sum(out=Vc[:, 2 * b:2 * b + 1],
                                     in_=NU[:, b * J2:(b + 1) * J2],
                                     axis=mybir.AxisListType.X)
                nc.vector.reduce_sum(out=Vc[:, 2 * b + 1:2 * b + 2],
                                     in_=BRBL[:, b * J2 + J:(b + 1) * J2],
                                     axis=mybir.AxisListType.X)
            OUT4 = pp.tile([2 * BPC, 1], f32)
            nc.tensor.matmul(out=OUT4[:], lhsT=Vc[:], rhs=ones24[:],
                             start=True, stop=True)
            outs = sb.tile([2 * BPC, 1], f32)
            nc.vector.tensor_copy(out=outs[:], in_=OUT4[:])
            nc.sync.dma_start(out=out_d, in_=outs[:])

    nc.compile()
    return nc


def get_program():
    if "nc" not in _CACHE:
        _CACHE["nc"] = _build_program()
    return _CACHE["nc"]


def make_in_maps(collision_idxs, vertices, faces, joint_regressor):
    """Host-side shard/layout prep. Returns list of per-core input dicts."""
    from ml_dtypes import bfloat16

    collision_idxs = np.asarray(collision_idxs)
    vertices = np.asarray(vertices)
    faces = np.asarray(faces)
    joint_regressor = np.asarray(joint_regressor).astype(np.float32)

    # T48[v] = [S(v) | M(v)] rows, zero row at v=N
    jrT = np.zeros((N + 1, J), dtype=np.float32)
    jrT[:N] = joint_regressor.T
    T48 = np.concatenate([jrT, (jrT > 0).astype(np.float32)], axis=1)
    T48 = T48.astype(bfloat16)                       # [N+1, 48]

    # jr16 / verts padded for the joints matmul
    jr16 = np.zeros((NPAD, J), dtype=bfloat16)
    jr16[:N] = joint_regressor.T.astype(bfloat16)

    fi32 = faces.astype(np.int32)                    # [B, F, 3]
    cfi = collision_idxs.astype(np.int32)            # [B, C, 2]

    in_maps = []
    for c in range(NCORES):
        bs = slice(c * BPC, (c + 1) * BPC)
        # TF table: [BPC*FPAD, 256] bf16, row b*FPAD+f = T48 of face f's
        # 3 vertices; f=F row is all-zero (T48[N] = 0 via vertex id N)
        fpad = np.full((BPC, FPAD, 3), N, dtype=np.int32)
        fpad[:, :F, :] = fi32[bs]
        tf = np.zeros((BPC, FPAD, EROW), dtype=bfloat16)
        tf[:, :, :3 * J2] = T48[fpad].reshape(BPC, FPAD, 3 * J2)

        # gather indices in wrapped int16 layout
        cfc = cfi[bs]                                # [BPC, C, 2]
        valid = cfc[:, :, 0:1] >= 0
        sel = np.where(valid, np.clip(cfc, 0, F), F)  # invalid -> zero row
        offs = (np.arange(BPC, dtype=np.int32) * FPAD)[:, None, None]
        iv = (sel + offs).astype(np.int32)           # [BPC, C, 2]
        # L[q, b, s, nn, p] = iv[b, (4q+nn)*128+p, s]
        iv5 = iv.reshape(BPC, NQ, CHQ, 128, 2)       # b, q, nn, p, s
        L = np.transpose(iv5, (1, 0, 4, 2, 3)).reshape(-1)
        # WF[16g+qq, q*128+m] = L[q*NPQ + m*16 + qq]
        W = L.reshape(NQ, NPQ // 16, 16)
        Wt = np.transpose(W, (2, 0, 1)).reshape(16, NIDX // 16)
        wfa = np.tile(Wt, (8, 1)).astype(np.int16)

        # verts packed [128, 54, 6] -> [NPAD, 6]: row p*54+k, col 3b+d
        vp = np.zeros((BPC, NPAD, 3), dtype=np.float32)
        vp[:, :N, :] = vertices[bs]
        vp6 = np.concatenate([vp[0], vp[1]], axis=1).astype(bfloat16)

        in_maps.append({
            "tf": np.ascontiguousarray(tf.reshape(BPC * FPAD, EROW)),
            "wf": np.ascontiguousarray(wfa),
            "jr16": jr16,
            "vp6": np.ascontiguousarray(vp6),
        })
    return in_maps


def kernel(collision_idxs, vertices, faces, joint_regressor):
    from concourse.bass_utils import run_bass_kernel_spmd

    nc = get_program()
    in_maps = make_in_maps(collision_idxs, vertices, faces, joint_regressor)
    res = run_bass_kernel_spmd(nc, in_maps, core_ids=list(range(NCORES)))
    num = 0.0
    den = 0.0
    for r in res.results:
        o = np.asarray(r["out"], dtype=np.float64).reshape(-1)
        num += o[0] + o[2]
        den += o[1] + o[3]
    if den > 0:
        val = num / max(den, 1.0)
    else:
        val = 0.0
    return np.float32(val)


# revision 4
# speedup vs baseline: 4.6500x; 1.0419x over previous
"""Trainium2 Bass kernel for nn_CollisonToJointLoss.

Math restructure (avoids the 3K*J*J elementwise blowup):
  joint_regressor >= 0, so where both gathered scores are nonzero,
  |intr_s + recv_s| = intr_s + recv_s.  For one collision-vertex pair
  (u = intruder vertex, w = receiver vertex, batch b):

    sum_{i,j} mask*d*(s_u[i]+s_w[j]) = s_u^T D_b m_w + m_u^T D_b s_w
    sum_{i,j} mask                   = m_u^T (11^T) m_w
  with s_v = jr[v], m_v = (s_v > 0), D_b = pairwise joint distances.
  Summing over pairs becomes accumulated PE matmuls with rows [S|M]:
  ACC = [S|M]_intr^T @ [S|M]_recv; TR = S_i^T M_r, BL = M_i^T S_r,
  BR = M_i^T M_r.  num_b = <D_b, TR + BL>, den_b = sum(BR).

Single-stage gather: the host joins faces with the jr score table into a
per-(batch,face) row TF[b*FPAD+f] = [T(v0)|T(v1)|T(v2)|pad] (fp8e4,
256B rows, T(v) = [S(24)|0(8)|M(24)|0(8)], 32-aligned halves so the
BL/BR blocks land on legal matmul tile positions), with an all-zero row
at f=F for invalid collisions.  One descriptor per collision side
(2*C*BPC = 8192 per core).  Gather indices are index arithmetic on
collision_idxs, so the host computes them in the wrapped int16 layout
dma_gather wants; the kernel just loads them.

fp8 + MatmulPerfMode.DoubleRow contract 2 k-tiles per instruction:
48 ACC matmuls (vs 96) and 27 joints matmuls (vs 54), keeping PE.SEQ
(~100ns/instr issue cost) off the critical path.  The gather is split
into 7 calls with small first/last calls: desc-gen (994+0.34/desc on
Pool) pipelines under the previous call's DMA, and only a small matmul
tail follows the last call.  End: TR/BL/BR are weighted by D on DVE
reading PSUM directly; the [24,144] partial product ships to the host,
which does the final (all-reduce) sum.

Sharding: data-parallel over batch B: 8 cores x 2 batches.
"""

import numpy as np

B, C, N, F, J = 16, 2048, 6890, 13776, 24
NCORES = 8
BPC = B // NCORES          # batches per core
NPAD = 6912                # 128 * 54  (jr/verts padded with zero rows)
KCH = NPAD // 128          # 54 chunks for the joints matmul
FPAD = F + 1               # TF rows per batch incl. zero row
EROW = 256                 # TF row bytes (fp8 elems)
NCH = C // 128             # 16 collision chunks per batch
NIDX = BPC * C * 2         # 8192 gather descriptors per core
CALL_N = [1, 2, 3, 3, 3, 3, 1]   # collision chunks per gather call
NU_W = 144                 # output partial width (TR|BL|BR x 2 batches)

_CACHE = {}


def _build_program():
    import concourse.bass as bass
    import concourse.tile as tile
    from concourse import bacc, mybir
    from concourse.masks import make_identity

    f32 = mybir.dt.float32
    fp8 = mybir.dt.float8e4
    i16 = mybir.dt.int16
    Alu = mybir.AluOpType
    DR = mybir.MatmulPerfMode.DoubleRow

    nc = bacc.Bacc("TRN2", target_bir_lowering=False, debug=False,
                   num_swdge_queues=2)

    tf = nc.dram_tensor("tf", [BPC * FPAD, EROW], fp8,
                        kind="ExternalInput").ap()
    wf = nc.dram_tensor("wf", [128, NIDX // 16], i16,
                        kind="ExternalInput").ap()
    jv = nc.dram_tensor("jv", [NPAD, 32], fp8, kind="ExternalInput").ap()
    out_d = nc.dram_tensor("out", [J, NU_W], f32, kind="ExternalOutput").ap()

    ncum = np.cumsum([0] + CALL_N)

    with tile.TileContext(nc) as tc:
        with tc.tile_pool(name="sb", bufs=1) as sb, \
             tc.tile_pool(name="pp", bufs=1, space="PSUM") as pp:

            # ---- loads (WF call-0 slice first: it gates the gather) -------
            WF = sb.tile([128, NIDX // 16], i16)
            c0 = CALL_N[0] * 32                    # cols for call 0
            nc.sync.dma_start(out=WF[:, 0:c0], in_=wf[:, 0:c0])
            nc.sync.dma_start(out=WF[:, c0:], in_=wf[:, c0:])

            JV = sb.tile([128, KCH, 32], fp8)      # [jr(24)|v6(6)|pad]
            nc.sync.dma_start(out=JV[:].rearrange("p k d -> p (k d)"),
                              in_=jv.rearrange("(p k) d -> p (k d)", p=128))

            ident = sb.tile([128, 128], f32)
            make_identity(nc, ident[:])

            # ---- gather: 7 pipelined calls ---------------------------------
            U = sb.tile([128, 4 * NCH, EROW], fp8)
            for q in range(len(CALL_N)):
                nidx = CALL_N[q] * 512
                nc.gpsimd.dma_gather(
                    out_ap=U[:, ncum[q] * 4:ncum[q + 1] * 4, :], in_ap=tf,
                    idxs_ap=WF[:, ncum[q] * 32:ncum[q + 1] * 32],
                    num_idxs=nidx, num_idxs_reg=nidx, elem_size=EROW,
                    single_packet=False, queue_num=q % 2)

            # ---- joints = jr^T-chunks contracted with verts (fp8 2-row) ---
            J6p = pp.tile([J, 6], f32)
            for k in range(KCH // 2):
                nc.tensor.matmul(
                    out=J6p[:],
                    lhsT=JV[:, 2 * k:2 * k + 2, 0:J],
                    rhs=JV[:, 2 * k:2 * k + 2, J:J + 6],
                    start=(k == 0), stop=(k == KCH // 2 - 1), perf_mode=DR)
            j6 = sb.tile([J, 6], f32)
            nc.vector.tensor_copy(out=j6[:], in_=J6p[:])
            j6sq = sb.tile([J, 6], f32)
            nc.vector.tensor_mul(out=j6sq[:], in0=j6[:], in1=j6[:])

            ones3_24 = sb.tile([3, J], f32)
            nc.vector.memset(ones3_24[:], 1.0)

            # jt_b = joints_b^T [3, 24] at partition base 0
            jtp = pp.tile([3, 2 * J], f32)
            for b in range(BPC):
                nc.tensor.transpose(out=jtp[:, J * b:J * b + J],
                                    in_=j6[:, 3 * b:3 * b + 3],
                                    identity=ident[:J, :J])
            jt = sb.tile([3, 2 * J], f32)
            nc.vector.tensor_copy(out=jt[:], in_=jtp[:])
            sq = sb.tile([3, 2 * J], f32)
            nc.vector.tensor_mul(out=sq[:], in0=jt[:], in1=jt[:])
            jtm2 = sb.tile([3, 2 * J], f32)
            nc.vector.tensor_scalar_mul(out=jtm2[:], in0=jt[:], scalar1=-2.0)

            # ---- D2 = [D_0 | D_1] pairwise joint distances (parts 0-23) ---
            G24 = pp.tile([J, J], f32)
            D2 = sb.tile([J, BPC, J], f32)
            rc = sb.tile([J, BPC], f32)
            for b in range(BPC):
                nc.vector.reduce_sum(out=rc[:, b:b + 1],
                                     in_=j6sq[:, 3 * b:3 * b + 3],
                                     axis=mybir.AxisListType.X)
            for b in range(BPC):
                nc.tensor.matmul(out=G24[:], lhsT=jtm2[:, J * b:J * b + J],
                                 rhs=jt[:, J * b:J * b + J],
                                 start=True, stop=False)
                nc.tensor.matmul(out=G24[:], lhsT=ones3_24[:],
                                 rhs=sq[:, J * b:J * b + J],
                                 start=False, stop=True)
                nc.vector.tensor_scalar(out=D2[:, b], in0=G24[:],
                                        scalar1=rc[:, b:b + 1], scalar2=0.0,
                                        op0=Alu.add, op1=Alu.max)
                nc.scalar.activation(out=D2[:, b], in_=D2[:, b],
                                     func=mybir.ActivationFunctionType.Sqrt)

            # ---- ACC matmuls stream behind each gather call ----------------
            # U c-slot = n*4 + b*2 + s; row = [T0|T1|T2|pad], T = [S|0|M|0]
            # DoubleRow: t0+t1 of chunk n in one instr; t2 of chunks
            # (2j, 2j+1) in one instr.
            ACC = [pp.tile([64, 64], f32, name=f"ACC{b}") for b in range(BPC)]
            Un = U[:].rearrange("p (n cc) e -> p n cc e", cc=4)
            for q in range(len(CALL_N)):
                for n in range(ncum[q], ncum[q + 1]):
                    for b in range(BPC):
                        ci, cr = b * 2 + 1, b * 2
                        nc.tensor.matmul(
                            out=ACC[b][:],
                            lhsT=Un[:, n, ci, 0:128]
                                .rearrange("p (two f) -> p two f", two=2),
                            rhs=Un[:, n, cr, 0:128]
                                .rearrange("p (two f) -> p two f", two=2),
                            start=(n == 0), stop=False, perf_mode=DR)
                    if n % 2 == 1:
                        for b in range(BPC):
                            ci, cr = b * 2 + 1, b * 2
                            nc.tensor.matmul(
                                out=ACC[b][:],
                                lhsT=Un[:, n - 1:n + 1, ci, 128:192],
                                rhs=Un[:, n - 1:n + 1, cr, 128:192],
                                start=False, stop=(n == NCH - 1),
                                perf_mode=DR)

            # ---- final: weight TR/BL/BR by D, ship partials to host -------
            # ACCs rows 32:64 (BL/BR rows) -> partitions 0:32 via selector
            ACCs = sb.tile([64, BPC * 64], f32)
            for b in range(BPC):
                nc.vector.tensor_copy(out=ACCs[32:64, b * 64:(b + 1) * 64],
                                      in_=ACC[b][32:64, :])
            BRBLp = pp.tile([32, BPC * 64], f32)
            nc.tensor.matmul(out=BRBLp[:], lhsT=ident[32:64, 32:64],
                             rhs=ACCs[32:64, :], start=True, stop=True)

            NU = sb.tile([J, NU_W], f32)
            NUv = NU[:].rearrange("p (g b c) -> p g b c", g=3, b=BPC)
            BLv = BRBLp[0:J, :].rearrange("p (b c) -> p b c", b=BPC)
            for b in range(BPC):   # TR = ACC[0:24, 32:56] (PSUM direct)
                nc.vector.tensor_mul(out=NUv[:, 0, b],
                                     in0=ACC[b][0:J, 32:32 + J],
                                     in1=D2[:, b])
            nc.vector.tensor_tensor(out=NUv[:, 1], in0=BLv[:, :, 0:J],
                                    in1=D2[:], op=Alu.mult)
            nc.vector.tensor_copy(out=NUv[:, 2], in_=BLv[:, :, 32:32 + J])
            nc.sync.dma_start(out=out_d, in_=NU[:])

    nc.compile()
    return nc


def get_program():
    if "nc" not in _CACHE:
        _CACHE["nc"] = _build_program()
    return _CACHE["nc"]


def make_in_maps(collision_idxs, vertices, faces, joint_regressor):
    """Host-side shard/layout prep. Returns list of per-core input dicts."""
    from ml_dtypes import float8_e4m3

    collision_idxs = np.asarray(collision_idxs)
    vertices = np.asarray(vertices)
    faces = np.asarray(faces)
    joint_regressor = np.asarray(joint_regressor).astype(np.float32)

    # T64[v] = [S(24)|0(8)|M(24)|0(8)] fp8 rows, zero row at v=N
    jrT = np.zeros((N + 1, J), dtype=np.float32)
    jrT[:N] = joint_regressor.T
    T64 = np.zeros((N + 1, 64), dtype=float8_e4m3)
    T64[:, 0:J] = jrT.astype(float8_e4m3)
    T64[:, 32:32 + J] = (jrT > 0).astype(float8_e4m3)

    fi32 = faces.astype(np.int32)                    # [B, F, 3]
    cfi = collision_idxs.astype(np.int32)            # [B, C, 2]

    in_maps = []
    for c in range(NCORES):
        bs = slice(c * BPC, (c + 1) * BPC)
        # TF table rows: [T64(v0)|T64(v1)|T64(v2)|pad(64)]
        fpad = np.full((BPC, FPAD, 3), N, dtype=np.int32)
        fpad[:, :F, :] = fi32[bs]
        tf = np.zeros((BPC, FPAD, EROW), dtype=float8_e4m3)
        tf[:, :, :192] = T64[fpad].reshape(BPC, FPAD, 192)

        # gather indices, wrapped int16: c-slot = n*4 + b*2 + s
        cfc = cfi[bs]                                # [BPC, C, 2]
        valid = cfc[:, :, 0:1] >= 0
        sel = np.where(valid, np.clip(cfc, 0, F), F)  # invalid -> zero row
        offs = (np.arange(BPC, dtype=np.int32) * FPAD)[:, None, None]
        iv = (sel + offs).reshape(BPC, NCH, 128, 2)  # b, n, p, s
        L = np.transpose(iv, (1, 0, 3, 2)).reshape(-1)   # (n, b, s, p)
        wfa = np.tile(L.reshape(NIDX // 16, 16).T, (8, 1)).astype(np.int16)

        # JV rows: [jr(24) | verts b0(3) b1(3) | pad(2)] fp8, row p*54+k
        jva = np.zeros((NPAD, 32), dtype=float8_e4m3)
        jva[:N, 0:J] = joint_regressor.T.astype(float8_e4m3)
        vs = vertices[bs]                            # [BPC, N, 3]
        vpad = np.zeros((BPC, NPAD, 3), dtype=np.float32)
        vpad[:, :N] = vs
        jva[:, J:J + 3] = vpad[0].astype(float8_e4m3)
        jva[:, J + 3:J + 6] = vpad[1].astype(float8_e4m3)

        in_maps.append({
            "tf": np.ascontiguousarray(tf.reshape(BPC * FPAD, EROW)),
            "wf": np.ascontiguousarray(wfa),
            "jv": np.ascontiguousarray(jva),
        })
    return in_maps


def kernel(collision_idxs, vertices, faces, joint_regressor):
    from concourse.bass_utils import run_bass_kernel_spmd

    nc = get_program()
    in_maps = make_in_maps(collision_idxs, vertices, faces, joint_regressor)
    res = run_bass_kernel_spmd(nc, in_maps, core_ids=list(range(NCORES)))
    num = 0.0
    den = 0.0
    for r in res.results:
        o = np.asarray(r["out"], dtype=np.float64)   # [J, 144]
        num += o[:, 0:96].sum()
        den += o[:, 96:144].sum()
    if den > 0:
        val = num / max(den, 1.0)
    else:
        val = 0.0
    return np.float32(val)


# revision 7
# speedup vs baseline: 4.7427x; 1.0199x over previous
"""Trainium2 Bass kernel for nn_CollisonToJointLoss.

Math restructure (avoids the 3K*J*J elementwise blowup):
  joint_regressor >= 0, so where both gathered scores are nonzero,
  |intr_s + recv_s| = intr_s + recv_s.  For one collision-vertex pair
  (u = intruder vertex, w = receiver vertex, batch b):

    sum_{i,j} mask*d*(s_u[i]+s_w[j]) = s_u^T D_b m_w + m_u^T D_b s_w
    sum_{i,j} mask                   = m_u^T (11^T) m_w
  with s_v = jr[v], m_v = (s_v > 0), D_b = pairwise joint distances.
  Summing over pairs becomes accumulated PE matmuls with rows [S|M]:
  ACC = [S|M]_intr^T @ [S|M]_recv; TR = S_i^T M_r, BL = M_i^T S_r,
  BR = M_i^T M_r.  num_b = <D_b, TR + BL>, den_b = sum(BR).

Single-stage gather: the host joins faces with the jr score table into a
per-(batch,face) row TF[b*FPAD+f] = [T(v0)|T(v1)|T(v2)|pad] (fp8e4,
256B rows, T(v) = [S(24)|0(8)|M(24)|0(8)], 32-aligned halves so the
BL/BR blocks land on legal matmul tile positions), with an all-zero row
at f=F for invalid collisions.  One descriptor per collision side
(2*C*BPC = 8192 per core).  Gather indices are index arithmetic on
collision_idxs, so the host computes them in the wrapped int16 layout
dma_gather wants; the kernel just loads them.

fp8 + MatmulPerfMode.DoubleRow contract 2 k-tiles per instruction:
48 ACC matmuls (vs 96) and 27 joints matmuls (vs 54), keeping PE.SEQ
(~100ns/instr issue cost) off the critical path.  The gather is split
into 7 calls with small first/last calls: desc-gen (994+0.34/desc on
Pool) pipelines under the previous call's DMA, and only a small matmul
tail follows the last call.  End: TR/BL/BR are weighted by D on DVE
reading PSUM directly; the [24,144] partial product ships to the host,
which does the final (all-reduce) sum.

Sharding: data-parallel over batch B: 8 cores x 2 batches.
"""

import numpy as np

B, C, N, F, J = 16, 2048, 6890, 13776, 24
NCORES = 8
BPC = B // NCORES          # batches per core
NPAD = 6912                # 128 * 54  (jr/verts padded with zero rows)
KCH = NPAD // 128          # 54 chunks for the joints matmul
FPAD = F + 1               # TF rows per batch incl. zero row
EROW = 256                 # TF row bytes (fp8 elems)
NCH = C // 128             # 16 collision chunks per batch
NIDX = BPC * C * 2         # 8192 gather descriptors per core
CALL_N = [2, 2, 3, 4, 3, 2]      # collision chunks per gather call
NU_W = 144                 # output partial width (TR|BL|BR x 2 batches)

_CACHE = {}


def _build_program():
    import concourse.bass as bass
    import concourse.tile as tile
    from concourse import bacc, mybir
    from concourse.masks import make_identity

    f32 = mybir.dt.float32
    fp8 = mybir.dt.float8e4
    i16 = mybir.dt.int16
    Alu = mybir.AluOpType
    DR = mybir.MatmulPerfMode.DoubleRow

    nc = bacc.Bacc("TRN2", target_bir_lowering=False, debug=False,
                   num_swdge_queues=2)

    tf = nc.dram_tensor("tf", [BPC * FPAD, EROW], fp8,
                        kind="ExternalInput").ap()
    wf = nc.dram_tensor("wf", [128, NIDX // 16], i16,
                        kind="ExternalInput").ap()
    jv = nc.dram_tensor("jv", [NPAD, 32], fp8, kind="ExternalInput").ap()
    out_d = nc.dram_tensor("out", [J, NU_W], f32, kind="ExternalOutput").ap()

    ncum = np.cumsum([0] + CALL_N)

    with tile.TileContext(nc) as tc:
        with tc.tile_pool(name="sb", bufs=1) as sb, \
             tc.tile_pool(name="pp", bufs=1, space="PSUM") as pp:

            # ---- loads (WF call-0 slice first: it gates the gather) -------
            WF = sb.tile([128, NIDX // 16], i16)
            c0 = CALL_N[0] * 32                    # cols for call 0
            nc.sync.dma_start(out=WF[:, 0:c0], in_=wf[:, 0:c0])
            nc.sync.dma_start(out=WF[:, c0:], in_=wf[:, c0:])

            JV = sb.tile([128, KCH, 32], fp8)      # [jr(24)|v6(6)|pad]
            nc.sync.dma_start(out=JV[:].rearrange("p k d -> p (k d)"),
                              in_=jv.rearrange("(p k) d -> p (k d)", p=128))

            ident = sb.tile([128, 128], f32)
            make_identity(nc, ident[:])

            # ---- gather: 7 pipelined calls ---------------------------------
            U = sb.tile([128, 4 * NCH, EROW], fp8)
            for q in range(len(CALL_N)):
                nidx = CALL_N[q] * 512
                nc.gpsimd.dma_gather(
                    out_ap=U[:, ncum[q] * 4:ncum[q + 1] * 4, :], in_ap=tf,
                    idxs_ap=WF[:, ncum[q] * 32:ncum[q + 1] * 32],
                    num_idxs=nidx, num_idxs_reg=nidx, elem_size=EROW,
                    single_packet=False, queue_num=q % 2)

            # ---- joints = jr^T-chunks contracted with verts (fp8 2-row) ---
            J6p = pp.tile([J, 6], f32)
            for k in range(KCH // 2):
                nc.tensor.matmul(
                    out=J6p[:],
                    lhsT=JV[:, 2 * k:2 * k + 2, 0:J],
                    rhs=JV[:, 2 * k:2 * k + 2, J:J + 6],
                    start=(k == 0), stop=(k == KCH // 2 - 1), perf_mode=DR)
            j6 = sb.tile([J, 6], f32)
            nc.vector.tensor_copy(out=j6[:], in_=J6p[:])
            j6sq = sb.tile([J, 6], f32)
            nc.vector.tensor_mul(out=j6sq[:], in0=j6[:], in1=j6[:])

            ones3_24 = sb.tile([3, J], f32)
            nc.vector.memset(ones3_24[:], 1.0)

            # jt_b = joints_b^T [3, 24] at partition base 0
            jtp = pp.tile([3, 2 * J], f32)
            for b in range(BPC):
                nc.tensor.transpose(out=jtp[:, J * b:J * b + J],
                                    in_=j6[:, 3 * b:3 * b + 3],
                                    identity=ident[:J, :J])
            jt = sb.tile([3, 2 * J], f32)
            nc.vector.tensor_copy(out=jt[:], in_=jtp[:])
            sq = sb.tile([3, 2 * J], f32)
            nc.vector.tensor_mul(out=sq[:], in0=jt[:], in1=jt[:])
            jtm2 = sb.tile([3, 2 * J], f32)
            nc.vector.tensor_scalar_mul(out=jtm2[:], in0=jt[:], scalar1=-2.0)

            # ---- D2 = [D_0 | D_1] pairwise joint distances (parts 0-23) ---
            G24 = pp.tile([J, J], f32)
            D2 = sb.tile([J, BPC, J], f32)
            rc = sb.tile([J, BPC], f32)
            for b in range(BPC):
                nc.vector.reduce_sum(out=rc[:, b:b + 1],
                                     in_=j6sq[:, 3 * b:3 * b + 3],
                                     axis=mybir.AxisListType.X)
            for b in range(BPC):
                nc.tensor.matmul(out=G24[:], lhsT=jtm2[:, J * b:J * b + J],
                                 rhs=jt[:, J * b:J * b + J],
                                 start=True, stop=False)
                nc.tensor.matmul(out=G24[:], lhsT=ones3_24[:],
                                 rhs=sq[:, J * b:J * b + J],
                                 start=False, stop=True)
                nc.vector.tensor_scalar(out=D2[:, b], in0=G24[:],
                                        scalar1=rc[:, b:b + 1], scalar2=0.0,
                                        op0=Alu.add, op1=Alu.max)
                nc.scalar.activation(out=D2[:, b], in_=D2[:, b],
                                     func=mybir.ActivationFunctionType.Sqrt)

            # ---- ACC matmuls stream behind each gather call ----------------
            # U c-slot = n*4 + b*2 + s; row = [T0|T1|T2|pad], T = [S|0|M|0]
            # DoubleRow: t0+t1 of chunk n in one instr; t2 of chunks
            # (2j, 2j+1) in one instr.
            ACC = [pp.tile([64, 64], f32, name=f"ACC{b}") for b in range(BPC)]
            Un = U[:].rearrange("p (n cc) e -> p n cc e", cc=4)
            for q in range(len(CALL_N)):
                for n in range(ncum[q], ncum[q + 1]):
                    for b in range(BPC):
                        ci, cr = b * 2 + 1, b * 2
                        nc.tensor.matmul(
                            out=ACC[b][:],
                            lhsT=Un[:, n, ci, 0:128]
                                .rearrange("p (two f) -> p two f", two=2),
                            rhs=Un[:, n, cr, 0:128]
                                .rearrange("p (two f) -> p two f", two=2),
                            start=(n == 0), stop=False, perf_mode=DR)
                    if n % 2 == 1:
                        for b in range(BPC):
                            ci, cr = b * 2 + 1, b * 2
                            nc.tensor.matmul(
                                out=ACC[b][:],
                                lhsT=Un[:, n - 1:n + 1, ci, 128:192],
                                rhs=Un[:, n - 1:n + 1, cr, 128:192],
                                start=False, stop=(n == NCH - 1),
                                perf_mode=DR)

            # ---- final: weight TR/BL/BR by D, ship partials to host -------
            # ACCs rows 32:64 (BL/BR rows) -> partitions 0:32 via selector
            ACCs = sb.tile([64, BPC * 64], f32)
            for b in range(BPC):
                nc.vector.tensor_copy(out=ACCs[32:64, b * 64:(b + 1) * 64],
                                      in_=ACC[b][32:64, :])
            BRBLp = pp.tile([32, BPC * 64], f32)
            nc.tensor.matmul(out=BRBLp[:], lhsT=ident[32:64, 32:64],
                             rhs=ACCs[32:64, :], start=True, stop=True)

            NU = sb.tile([J, NU_W], f32)
            NUv = NU[:].rearrange("p (g b c) -> p g b c", g=3, b=BPC)
            BLv = BRBLp[0:J, :].rearrange("p (b c) -> p b c", b=BPC)
            for b in range(BPC):   # TR = ACC[0:24, 32:56] (PSUM direct)
                nc.vector.tensor_mul(out=NUv[:, 0, b],
                                     in0=ACC[b][0:J, 32:32 + J],
                                     in1=D2[:, b])
            nc.vector.tensor_tensor(out=NUv[:, 1], in0=BLv[:, :, 0:J],
                                    in1=D2[:], op=Alu.mult)
            nc.vector.tensor_copy(out=NUv[:, 2], in_=BLv[:, :, 32:32 + J])
            nc.sync.dma_start(out=out_d, in_=NU[:])

    nc.compile()
    return nc


def get_program():
    if "nc" not in _CACHE:
        _CACHE["nc"] = _build_program()
    return _CACHE["nc"]


def make_in_maps(collision_idxs, vertices, faces, joint_regressor):
    """Host-side shard/layout prep. Returns list of per-core input dicts."""
    from ml_dtypes import float8_e4m3

    collision_idxs = np.asarray(collision_idxs)
    vertices = np.asarray(vertices)
    faces = np.asarray(faces)
    joint_regressor = np.asarray(joint_regressor).astype(np.float32)

    # T64[v] = [S(24)|0(8)|M(24)|0(8)] fp8 rows, zero row at v=N
    jrT = np.zeros((N + 1, J), dtype=np.float32)
    jrT[:N] = joint_regressor.T
    T64 = np.zeros((N + 1, 64), dtype=float8_e4m3)
    T64[:, 0:J] = jrT.astype(float8_e4m3)
    T64[:, 32:32 + J] = (jrT > 0).astype(float8_e4m3)

    fi32 = faces.astype(np.int32)                    # [B, F, 3]
    cfi = collision_idxs.astype(np.int32)            # [B, C, 2]

    in_maps = []
    for c in range(NCORES):
        bs = slice(c * BPC, (c + 1) * BPC)
        # TF table rows: [T64(v0)|T64(v1)|T64(v2)|pad(64)]
        fpad = np.full((BPC, FPAD, 3), N, dtype=np.int32)
        fpad[:, :F, :] = fi32[bs]
        tf = np.zeros((BPC, FPAD, EROW), dtype=float8_e4m3)
        tf[:, :, :192] = T64[fpad].reshape(BPC, FPAD, 192)

        # gather indices, wrapped int16: c-slot = n*4 + b*2 + s
        cfc = cfi[bs]                                # [BPC, C, 2]
        valid = cfc[:, :, 0:1] >= 0
        sel = np.where(valid, np.clip(cfc, 0, F), F)  # invalid -> zero row
        offs = (np.arange(BPC, dtype=np.int32) * FPAD)[:, None, None]
        iv = (sel + offs).reshape(BPC, NCH, 128, 2)  # b, n, p, s
        L = np.transpose(iv, (1, 0, 3, 2)).reshape(-1)   # (n, b, s, p)
        wfa = np.tile(L.reshape(NIDX // 16, 16).T, (8, 1)).astype(np.int16)

        # JV rows: [jr(24) | verts b0(3) b1(3) | pad(2)] fp8, row p*54+k
        jva = np.zeros((NPAD, 32), dtype=float8_e4m3)
        jva[:N, 0:J] = joint_regressor.T.astype(float8_e4m3)
        vs = vertices[bs]                            # [BPC, N, 3]
        vpad = np.zeros((BPC, NPAD, 3), dtype=np.float32)
        vpad[:, :N] = vs
        jva[:, J:J + 3] = vpad[0].astype(float8_e4m3)
        jva[:, J + 3:J + 6] = vpad[1].astype(float8_e4m3)

        in_maps.append({
            "tf": np.ascontiguousarray(tf.reshape(BPC * FPAD, EROW)),
            "wf": np.ascontiguousarray(wfa),
            "jv": np.ascontiguousarray(jva),
        })
    return in_maps


def kernel(collision_idxs, vertices, faces, joint_regressor):
    from concourse.bass_utils import run_bass_kernel_spmd

    nc = get_program()
    in_maps = make_in_maps(collision_idxs, vertices, faces, joint_regressor)
    res = run_bass_kernel_spmd(nc, in_maps, core_ids=list(range(NCORES)))
    num = 0.0
    den = 0.0
    for r in res.results:
        o = np.asarray(r["out"], dtype=np.float64)   # [J, 144]
        num += o[:, 0:96].sum()
        den += o[:, 96:144].sum()
    if den > 0:
        val = num / max(den, 1.0)
    else:
        val = 0.0
    return np.float32(val)
